# revision 35
# baseline (speedup 1.0000x reference)
"""Multi-head attention (B=1, S=2048, H=1024, NH=16) on 8 trn2 NeuronCores.

Sharding: head-parallel. Core c owns heads {2c, 2c+1} (= 128 of the 1024
hidden dims). Each core computes its Q/K/V projection slices, the full
attention for its 2 heads, and a full-width partial of the output
projection (contraction over its 128 context dims). Host sums the 8
partials and adds the (host-folded) biases.

Masked-softmax restructure: the reference zeroes masked scores before
softmax, i.e. the numerator is m*e + (1-m) with e = exp(s/8). Using
m*e + (1-m) = m*(e-1) + 1, the kernel computes
  et = (exp(s/8) - 1) * m          (Act exp from PSUM; DVE sub at 4x,
                                    DVE mult at 2x with a bf16 mask)
and folds the "+1" into the PV matmul as a host-precomputed column-sum
of V (3 tiny rank-1 matmuls per head add colsum(V)+count to PSUM).

Engine budget per core (TimelineSim cost model):
  PE  ~73us  S (27) + PV (15) + QKVO projections (28) + transposes
  Act ~58us  exp only (reads score PSUM directly)
  DVE ~62us  (e-1)*m + normalize + tp copies + q-proj eviction
  Pool ~45us v-proj/k-proj/y evictions (idle engine in baseline)
  DMA ~70us  q,k,v 12MB + mask 8MB (bf16 for the 2x TT) + w 1MB + y 4MB
"""

import math

import numpy as np
import ml_dtypes

BF16 = ml_dtypes.bfloat16
FP8 = ml_dtypes.float8_e4m3
S, H, NH, DK = 2048, 1024, 16, 64
NCORES = 8
HPC = NH // NCORES          # heads per core = 2
DPC = HPC * DK              # head dims per core = 128
KC = H // 128               # contraction chunks = 8
TP = S // 512               # 512-wide token panels = 4
JC = S // 128               # 128-wide key chunks = 16
VA = DK + 1                 # v columns + ones column = 65

_CACHE = {}


def _oslc(ic):
    """o_ps column offset for ic-th 65-wide slice: 7 slices per 512-fp32
    PSUM bank so no matmul crosses a bank boundary."""
    b, r = divmod(ic, 7)
    return b * 512 + r * VA


def _build_program():
    """Build + compile the (identical) per-core Bass program."""
    from contextlib import ExitStack

    import concourse.bacc as bacc
    import concourse.tile as tile
    from concourse import mybir

    dt = mybir.dt
    AF = mybir.ActivationFunctionType
    f8 = dt.float8e4

    nc = bacc.Bacc("TRN2", target_bir_lowering=False, debug=False)

    # token-quarter-major x layouts: [4q][128 p][8 c][512 i] flattened
    qx_d = nc.dram_tensor("qx", [4 * 128, KC * 512], dt.bfloat16, kind="ExternalInput").ap()
    kx_d = nc.dram_tensor("kx", [4 * 128, KC * 512], dt.bfloat16, kind="ExternalInput").ap()
    vx_d = nc.dram_tensor("vx", [4 * 128, KC * 512], dt.bfloat16, kind="ExternalInput").ap()
    maskT_d = nc.dram_tensor("maskT", [S, S], dt.float8e4, kind="ExternalInput").ap()
    wq_d = nc.dram_tensor("wq", [128, KC * DPC], dt.bfloat16, kind="ExternalInput").ap()
    wk_d = nc.dram_tensor("wk", [128, KC * DPC], dt.bfloat16, kind="ExternalInput").ap()
    # wv | wo | ident packed: one DMA for the non-critical weights
    wpk_d = nc.dram_tensor("wpk", [128, 2 * H + 128], dt.bfloat16, kind="ExternalInput").ap()
    bqk_d = nc.dram_tensor("bqk", [DPC, 2], dt.float32, kind="ExternalInput").ap()
    vcr_d = nc.dram_tensor("vcr", [1, HPC * 7 * VA], dt.bfloat16, kind="ExternalInput").ap()
    yT_d = nc.dram_tensor("yT", [H, S], dt.bfloat16, kind="ExternalOutput").ap()

    with tile.TileContext(nc) as tc, ExitStack() as ctx:
        cp = ctx.enter_context(tc.tile_pool(name="const", bufs=1))
        e_p = ctx.enter_context(tc.tile_pool(name="ex", bufs=4))
        ot_p = ctx.enter_context(tc.tile_pool(name="otok", bufs=2))
        rc_p = ctx.enter_context(tc.tile_pool(name="recip", bufs=3))

        # ---- DMA priority: wq, qx quarters (PE-critical), rest behind ----
        wq_sb = cp.tile([128, KC * DPC], dt.bfloat16, tag="wq")
        nc.sync.dma_start(out=wq_sb, in_=wq_d)
        ones_col = cp.tile([1, 128], dt.bfloat16, tag="ones")
        nc.vector.memset(ones_col, 1.0)
        warm = cp.tile([128, 512], dt.bfloat16, tag="warm")
        nc.vector.memset(warm, 0.0)

        qT_sb = cp.tile([128, S], dt.bfloat16, tag="qTs")
        kT_sb = cp.tile([128, S], dt.bfloat16, tag="kTs")
        vaug = cp.tile([128, JC * (HPC * VA)], dt.bfloat16, tag="vaug")
        oT_sb = [cp.tile([128, 512], dt.bfloat16, tag=f"oTp{p}", name=f"oTp{p}")
                 for p in range(TP)]
        # y pair-tiles: cols 0:512 = even nn, 512:1024 = odd nn (one panel)
        y_p = ctx.enter_context(tc.tile_pool(name="ysb", bufs=2))

        import concourse.bass as bass_mod

        # tiny inputs first: they gate the projection evictions
        bqk_sb = cp.tile([DPC, 2], dt.float32, tag="bqk")
        nc.sync.dma_start(out=bqk_sb, in_=bqk_d)
        bq_sb = bqk_sb[:, 0:1]
        bk_sb = bqk_sb[:, 1:2]
        vcr_sb = cp.tile([1, HPC * 7 * VA], dt.bfloat16, tag="vcr")
        nc.sync.dma_start(out=vcr_sb, in_=vcr_d)
        etab = cp.tile([1, 2], dt.bfloat16, tag="etab")
        nc.scalar.activation(etab, ones_col[:, 0:2], AF.Exp)

        kv_pool = ctx.enter_context(tc.tile_pool(name="kvin", bufs=1))
        kin = [None] * TP
        vin = [None] * TP

        def x_quarter(which, qq):
            d, store, pre = {
                "k": (kx_d, kin, "k"), "v": (vx_d, vin, "v"),
            }[which]
            t_ = kv_pool.tile([128, KC * 512], dt.bfloat16, tag=f"x{pre}{qq}",
                              name=f"x{pre}{qq}")
            nc.sync.dma_start(out=t_, in_=d[qq * 128:(qq + 1) * 128, :])
            store[qq] = t_

        with tc.tile_pool(name="qxin", bufs=2) as qx_pool, \
             tc.tile_pool(name="ps_proj", bufs=1, space="PSUM") as pq:
            qin = [None] * TP

            def q_quarter(qq):
                t_ = qx_pool.tile([128, KC * 512], dt.bfloat16, tag="xq",
                                  name=f"xq{qq}")
                nc.sync.dma_start(out=t_, in_=qx_d[qq * 128:(qq + 1) * 128, :])
                qin[qq] = t_

            q_quarter(0)
            q_quarter(1)
            wk_sb = cp.tile([128, KC * DPC], dt.bfloat16, tag="wk")
            nc.sync.dma_start(out=wk_sb, in_=wk_d)
            x_quarter("k", 0)
            q_quarter(2)
            q_quarter(3)
            wpk = cp.tile([128, 2 * H + 128], dt.bfloat16, tag="wpk")
            nc.sync.dma_start(out=wpk, in_=wpk_d)
            w_sb = {"wq": wq_sb, "wk": wk_sb, "wv": wpk[:, 0:H]}
            wo_sb = wpk[:, H:2 * H]
            ident = wpk[:, 2 * H:2 * H + 128]
            # warm up the PE p-state ramp while the first q quarter lands
            wps = pq.tile([128, 512], dt.float32, tag="warmp", name="warmp")
            for i in range(10):
                nc.tensor.matmul(
                    wps, lhsT=warm[:, 0:128], rhs=warm,
                    start=(i == 0), stop=(i == 9),
                )
            # vaug ones-columns: one strided memset (col 64 of every 65-slice)
            vaug_ones = bass_mod.AP(
                tensor=vaug.tensor,
                offset=vaug.offset + DK,
                ap=[vaug.ap[0], [VA, JC * HPC]],
            )
            nc.vector.memset(vaug_ones, 1.0)

            def q_proj_panel(p):
                ps = pq.tile([128, 512], dt.float32, tag=f"pq{p}", name=f"pq{p}")
                for kk in range(KC):
                    nc.tensor.matmul(
                        ps,
                        lhsT=w_sb["wq"][:, kk * DPC:(kk + 1) * DPC],
                        rhs=qin[p][:, kk * 512:(kk + 1) * 512],
                        start=(kk == 0),
                        stop=(kk == KC - 1),
                    )
                nc.vector.tensor_scalar_add(
                    qT_sb[:, p * 512:(p + 1) * 512], ps, bq_sb
                )

            kps = pq.tile([128, 512], dt.float32, tag="pk0", name="pk0")

            def k0_proj():
                for kk in range(KC):
                    nc.tensor.matmul(
                        kps,
                        lhsT=w_sb["wk"][:, kk * DPC:(kk + 1) * DPC],
                        rhs=kin[0][:, kk * 512:(kk + 1) * 512],
                        start=(kk == 0),
                        stop=(kk == KC - 1),
                    )
                nc.vector.tensor_scalar_add(kT_sb[:, 0:512], kps, bk_sb)

            q_proj_panel(0)
            q_proj_panel(1)
            k0_proj()

        # ---- mask: fp8 DMA into the upper byte-half of each block's
        # bf16 slot; the idle Pool engine converts in place (the write
        # pointer trails the read pointer, so forward streaming is safe).
        # h0's TT reads the fp8 view at DVE 1x (h0 is DMA-paced anyway);
        # h1 reads the converted bf16 at 2x. The conversion is emitted at
        # h0's iteration j, after h0's TT(j) consumed the fp8 view. ----
        mask_lo = cp.tile([128, 8 * S], dt.bfloat16, tag="mask_lo")
        mask_hi = cp.tile([128, 8 * S], dt.bfloat16, tag="mask_hi")

        def mask_ap(j):
            t_ = mask_lo if j < 8 else mask_hi
            return t_[:, (j % 8) * S:(j % 8 + 1) * S]

        def mask8_ap(j):
            t_ = (mask_lo if j < 8 else mask_hi).bitcast(f8)
            return t_[:, (j % 8) * 2 * S + S:(j % 8) * 2 * S + 2 * S]

        def mask_pair(j):
            t_ = (mask_lo if j < 8 else mask_hi).bitcast(f8)
            nc.sync.dma_start(
                out=t_[:, (j % 8) * 2 * S + S:(j % 8 + 2) * 2 * S].rearrange(
                    "p (a i) -> p a i", a=2
                )
                if False else bass_mod.AP(
                    tensor=t_.tensor,
                    offset=t_.offset + (j % 8) * 2 * S + S,
                    ap=[t_.ap[0], [2 * S, 2], [1, S]],
                ),
                in_=maskT_d[j * 128:(j + 2) * 128, :].rearrange(
                    "(a p) i -> p a i", p=128
                ),
            )

        for tok in ("m0 k1 v0 m2 m4 v1 k2 m6 m8 v2 k3 m10 m12 v3 "
                    "m14").split():
            if tok[0] == "m":
                mask_pair(int(tok[1:]))
            else:
                x_quarter(tok[0], int(tok[1:]))

        # ---- attention; V projection and just-in-time K-panel projections
        # ride the h=0 j-loop. PSUM banks: s 2x2 + o 3 + misc 1 = 8
        if True:
            with tc.tile_pool(name="ps_misc", bufs=1, space="PSUM") as pm, \
                 tc.tile_pool(name="ps_s", bufs=2, space="PSUM") as ps_p, \
                 tc.tile_pool(name="ps_o", bufs=1, space="PSUM") as po_p:

                def q_proj_late(p):
                    ps = pm.tile([128, 512], dt.float32, tag="misc",
                                 name=f"pq{p}")
                    for kk in range(KC):
                        nc.tensor.matmul(
                            ps[:, 0:512],
                            lhsT=w_sb["wq"][:, kk * DPC:(kk + 1) * DPC],
                            rhs=qin[p][:, kk * 512:(kk + 1) * 512],
                            start=(kk == 0),
                            stop=(kk == KC - 1),
                        )
                    nc.vector.tensor_scalar_add(
                        qT_sb[:, p * 512:(p + 1) * 512], ps[:, 0:512], bq_sb
                    )

                def k_proj_panel(p):
                    ps = pm.tile([128, 512], dt.float32, tag="misc", name=f"pk{p}")
                    for kk in range(KC):
                        nc.tensor.matmul(
                            ps[:, 0:512],
                            lhsT=w_sb["wk"][:, kk * DPC:(kk + 1) * DPC],
                            rhs=kin[p][:, kk * 512:(kk + 1) * 512],
                            start=(kk == 0),
                            stop=(kk == KC - 1),
                        )
                    nc.vector.tensor_scalar_add(
                        kT_sb[:, p * 512:(p + 1) * 512], ps[:, 0:512], bk_sb
                    )

                def v_proj_chunk(t):
                    """Token-chunk t of the V projection into vaug (Act evicts)."""
                    ps = pm.tile([128, 512], dt.float32, tag="misc", name=f"pv{t}")
                    qq, ts_ = divmod(t, 4)
                    for kk in range(KC):
                        nc.tensor.matmul(
                            ps[:, 0:DPC],
                            lhsT=vin[qq][:, kk * 512 + ts_ * 128: kk * 512 + (ts_ + 1) * 128],
                            rhs=w_sb["wv"][:, kk * DPC:(kk + 1) * DPC],
                            start=(kk == 0),
                            stop=(kk == KC - 1),
                        )
                    base = t * (HPC * VA)
                    vdst = bass_mod.AP(
                        tensor=vaug.tensor,
                        offset=vaug.offset + base,
                        ap=[vaug.ap[0], [VA, HPC], [1, DK]],
                    )
                    nc.scalar.copy(vdst, ps[:, 0:DPC].rearrange(
                        "p (a d) -> p a d", a=HPC))

                def pv_mms(h, j, et, o_ps):
                    for ic in range(JC):
                        nc.tensor.matmul(
                            o_ps[:, _oslc(ic): _oslc(ic) + VA],
                            lhsT=et[:, ic * 128:(ic + 1) * 128],
                            rhs=vaug[:, j * (HPC * VA) + h * VA: j * (HPC * VA) + (h + 1) * VA],
                            start=(j == 0 and ic % 7 == 0),
                            stop=False,
                        )

                deferred = []  # [hs, ot_big, next_ic] transpose drip

                def drip_one():
                    if not deferred or deferred[0][2] >= JC:
                        return
                    dhs, dot, ic = deferred[0]
                    deferred[0][2] += 1
                    tp = pm.tile([DK, 128], dt.bfloat16, tag="misc",
                                 name=f"dtp{ic}")
                    nc.tensor.transpose(tp, dot[:, ic * DK:(ic + 1) * DK], ident)
                    nc.vector.tensor_copy(
                        oT_sb[ic // 4][dhs:dhs + DK,
                                       (ic % 4) * 128:(ic % 4 + 1) * 128],
                        tp,
                    )

                def epilogue(h, o_ps):
                    hs = h * DK
                    # +1 correction: colsum(V)+count via rank-1 matmuls,
                    # one per 65-wide slice (stop closes each PSUM bank)
                    for ic in range(JC):
                        nc.tensor.matmul(
                            o_ps[:, _oslc(ic): _oslc(ic) + VA],
                            lhsT=ones_col,
                            rhs=vcr_sb[:, h * 7 * VA + (ic % 7) * VA:
                                       h * 7 * VA + (ic % 7) * VA + VA],
                            start=False,
                            stop=(ic in (6, 13, 15)),
                        )
                    ot_big = ot_p.tile([128, JC * DK], dt.bfloat16, tag="ot")
                    if h < HPC - 1:
                        deferred.append([hs, ot_big, 0])
                    for b in range(3):
                        n_ic = (7, 7, 2)[b]
                        rc = rc_p.tile([128, 8], dt.float32, tag="rc",
                                       name=f"rc{h}_{b}")
                        den = bass_mod.AP(
                            tensor=o_ps.tensor,
                            offset=o_ps.offset + b * 512 + DK,
                            ap=[o_ps.ap[0], [VA, n_ic]],
                        )
                        nc.vector.reciprocal(rc[:, :n_ic], den)
                        src_ap = bass_mod.AP(
                            tensor=o_ps.tensor,
                            offset=o_ps.offset + b * 512,
                            ap=[o_ps.ap[0], [VA, n_ic], [1, DK]],
                        )
                        rcb = bass_mod.AP(
                            tensor=rc.tensor,
                            offset=rc.offset,
                            ap=[rc.ap[0], [1, n_ic], [0, DK]],
                        )
                        dst = ot_big[:, b * 7 * DK:(b * 7 + n_ic) * DK].rearrange(
                            "p (a d) -> p a d", d=DK
                        )
                        nc.vector.tensor_mul(dst, src_ap, rcb)
                        if h < HPC - 1:
                            continue
                        # tail: transpose/copy this bank's slices
                        for ic in range(b * 7, b * 7 + n_ic):
                            ot = ot_big[:, ic * DK:(ic + 1) * DK]
                            if ic % 2 == 0:
                                tp = ps_p.tile([DK, 128], dt.bfloat16,
                                               tag="sps", name=f"tp{h}_{ic}")
                            else:
                                tp = pm.tile([DK, 128], dt.bfloat16,
                                             tag="misc", name=f"tp{h}_{ic}")
                            nc.tensor.transpose(tp, ot, ident)
                            dsto = oT_sb[ic // 4][hs:hs + DK,
                                                  (ic % 4) * 128:(ic % 4 + 1) * 128]
                            if ic % 2 == 0:
                                nc.vector.tensor_copy(dsto, tp)
                            else:
                                nc.scalar.copy(dsto, tp)
                        for p in ((0,), (1, 2), (3,))[b]:
                            y_panel(p)

                def y_panel(p):
                    # O-projection: 4 pair-matmuls per panel into a 3-deep
                    # rotation of PSUM pair-buffers (2x "sps" + 1 slice of
                    # the idle o banks); evictions split across Act and DVE
                    for pr in range(4):
                        gp = p * 4 + pr
                        if gp % 3 == 2:
                            yb = po_p.tile([128, 1536], dt.float32, tag="ops",
                                           name=f"ybig{gp}")
                            y_ps = yb[:, 0:1024]
                        else:
                            y_ps = ps_p.tile([128, 1024], dt.float32,
                                             tag="sps", name=f"y{p}_{pr}")
                        y_t = y_p.tile([128, 1024], dt.bfloat16,
                                       tag=f"ysb{pr}", name=f"y_t{p}_{pr}")
                        for e in range(2):
                            nc.tensor.matmul(
                                y_ps[:, e * 512:(e + 1) * 512],
                                lhsT=wo_sb[:, (2 * pr + e) * 128:
                                           (2 * pr + e + 1) * 128],
                                rhs=oT_sb[p],
                                start=True,
                                stop=True,
                            )
                        nc.scalar.copy(y_t[:, 0:512], y_ps[:, 0:512])
                        nc.vector.tensor_copy(y_t[:, 512:1024],
                                              y_ps[:, 512:1024])
                        nc.sync.dma_start(
                            out=yT_d[2 * pr * 128:(2 * pr + 2) * 128,
                                     p * 512:(p + 1) * 512
                                     ].rearrange("(a p) i -> p a i", p=128),
                            in_=y_t.rearrange("p (a i) -> p a i", a=2),
                        )

                # flat 32-step stream over (head, key-block): the software
                # pipeline (depth 2) carries across the head boundary so
                # h1's S/exp/mask run while h0's epilogue drains
                pend = []
                o_cur = [None, None]  # (head, o_ps tile)
                for gi in range(HPC * JC + 2):
                    if len(pend) == 2 or gi >= HPC * JC:
                        hd, jd, etd = pend.pop(0)
                        if o_cur[0] != hd:
                            o_cur = [hd, po_p.tile([128, 1536], dt.float32,
                                                   tag="ops", name=f"ops{hd}")]
                        if hd == 0:
                            v_proj_chunk(jd)
                        pv_mms(hd, jd, etd, o_cur[1])
                        if jd == JC - 1:
                            epilogue(hd, o_cur[1])
                    if gi >= HPC * JC:
                        continue
                    h, j = divmod(gi, JC)
                    hs = h * DK
                    if h == 0 and j in (4, 8, 12):
                        k_proj_panel(j // 4)
                    et = e_p.tile([128, S], dt.bfloat16, tag="et")
                    s_tiles = []
                    for half in range(2):
                        s_ps = ps_p.tile([128, 1024], dt.float32, tag="sps")
                        s_tiles.append(s_ps)
                        for q in range(2):
                            pi = half * 2 + q
                            nc.tensor.matmul(
                                s_ps[:, q * 512:(q + 1) * 512],
                                lhsT=kT_sb[hs:hs + DK, j * 128:(j + 1) * 128],
                                rhs=qT_sb[hs:hs + DK, pi * 512:(pi + 1) * 512],
                                start=True,
                                stop=True,
                            )
                        if gi == 0 and half == 0:
                            # late q panels: S(j0,half0) only needs panels
                            # 0-1, so panels 2-3 project while it runs
                            q_proj_late(2)
                            q_proj_late(3)
                    a, bt = s_tiles
                    if bt.offset == a.offset + 1024 and bt.tensor is a.tensor:
                        # adjacent ring slots: one 2048-wide exp + sub
                        s_all = bass_mod.AP(
                            tensor=a.tensor, offset=a.offset,
                            ap=[a.ap[0], [1, 2048]],
                        )
                        nc.scalar.activation(et, s_all, AF.Exp,
                                             scale=1.0 / math.sqrt(DK))
                        nc.vector.tensor_scalar_sub(et, et, 1.0)
                    else:
                        for half in range(2):
                            eh = et[:, half * 1024:(half + 1) * 1024]
                            nc.scalar.activation(eh, s_tiles[half], AF.Exp,
                                                 scale=1.0 / math.sqrt(DK))
                            nc.vector.tensor_scalar_sub(eh, eh, 1.0)
                    if h == 0:
                        nc.vector.tensor_mul(et, et, mask8_ap(j))
                        nc.gpsimd.tensor_copy(mask_ap(j), mask8_ap(j))
                    else:
                        nc.vector.tensor_mul(et, et, mask_ap(j))

                    if h == 1:
                        drip_one()
                        if gi >= 30:
                            drip_one()
                    pend.append((h, j, et))

    nc.compile()
    return nc


def get_program():
    if "nc" not in _CACHE:
        _CACHE["nc"] = _build_program()
    return _CACHE["nc"]


def _wshuf(wT):
    """[1024 k, 128 n] -> [128 p, KC*128] with chunk kk at cols kk*128."""
    return np.ascontiguousarray(
        wT.reshape(KC, 128, DPC).transpose(1, 0, 2).reshape(128, KC * DPC)
    ).astype(BF16)


def _xquarters(x):
    """[S tok, H feat] fp32 -> [4*128, 8*512] bf16 token-quarter-major:
    [q][p][c][i] with element = x[512q + i, 128c + p]."""
    xT = np.asarray(x, np.float32).T            # [H, S]
    x4 = xT.reshape(KC, 128, TP, 512)           # [c, p, q, i]
    return np.ascontiguousarray(
        x4.transpose(2, 1, 0, 3).reshape(TP * 128, KC * 512)
    ).astype(BF16)


def make_in_maps(query, key, value, attention_mask, Wq, bq, Wk, bk, Wv, Wo):
    """Host-side sharding: per-core input dicts."""
    qx = _xquarters(np.asarray(query, np.float32)[0])
    kx = _xquarters(np.asarray(key, np.float32)[0])
    vx = _xquarters(np.asarray(value, np.float32)[0])
    maskT = np.ascontiguousarray(
        np.asarray(attention_mask, np.float32)[0, 0].T
    ).astype(FP8)
    # colsum(V)+count per core-head, device-matched: colsum over bf16(V)
    v_f = np.asarray(value, np.float32)[0]
    Wv_f = np.asarray(Wv, np.float32)

    in_maps = []
    for c in range(NCORES):
        ns = slice(c * DPC, (c + 1) * DPC)
        vproj = (v_f @ Wv_f[ns].T).astype(BF16).astype(np.float32)  # [S, 128]
        vcol = vproj.sum(axis=0)                                    # [128]
        vcr = np.zeros((1, HPC * 7 * VA), np.float32)
        for h in range(HPC):
            tile65 = np.concatenate([vcol[h * DK:(h + 1) * DK], [float(S)]])
            vcr[0, h * 7 * VA:(h + 1) * 7 * VA] = np.tile(tile65, 7)
        wpk = np.concatenate(
            [
                _wshuf(Wv_f[ns].T),
                np.ascontiguousarray(np.asarray(Wo, np.float32)[:, ns].T).astype(BF16),
                np.eye(128, dtype=BF16),
            ],
            axis=1,
        )
        bqk = np.stack(
            [np.asarray(bq, np.float32)[ns], np.asarray(bk, np.float32)[ns]],
            axis=1,
        )
        in_maps.append(
            {
                "qx": qx,
                "kx": kx,
                "vx": vx,
                "maskT": maskT,
                "wq": _wshuf(np.asarray(Wq, np.float32)[ns].T),
                "wk": _wshuf(np.asarray(Wk, np.float32)[ns].T),
                "wpk": np.ascontiguousarray(wpk),
                "bqk": np.ascontiguousarray(bqk),
                "vcr": vcr.astype(BF16),
            }
        )
    return in_maps


def combine_outputs(results, Wv_bias, Wo, bo):
    """Sum per-core partial yT's (bf16 -> fp32), add host-folded biases."""
    acc = np.zeros((H, S), np.float32)
    for r in results:
        acc += r["yT"].astype(np.float32)
    bias = np.asarray(bo, np.float32) + np.asarray(Wv_bias, np.float32) @ np.asarray(
        Wo, np.float32
    ).T
    return (acc.T + bias[None, :]).astype(np.float32)[None]


def kernel(
    query,
    key,
    value,
    attention_mask,
    Wq,
    bq,
    Wk,
    bk,
    Wv,
    bv,
    Wo,
    bo,
    head,
    hidden_size,
):
    from concourse.bass_utils import run_bass_kernel_spmd

    nc = get_program()
    in_maps = make_in_maps(
        query, key, value, attention_mask, Wq, bq, Wk, bk, Wv, Wo
    )
    res = run_bass_kernel_spmd(nc, in_maps, list(range(NCORES)))
    return combine_outputs(res.results, bv, Wo, bo)


# revision 36
# speedup vs baseline: 1.0542x; 1.0542x over previous
"""Multi-head attention (B=1, S=2048, H=1024, NH=16) on 8 trn2 NeuronCores.

Sharding: head-parallel. Core c owns heads {2c, 2c+1} (= 128 of the 1024
hidden dims). Each core computes its Q/K/V projection slices, the full
attention for its 2 heads, and a full-width partial of the output
projection (contraction over its 128 context dims). Host sums the 8
partials and adds the (host-folded) biases.

Masked-softmax restructure: the reference zeroes masked scores before
softmax, i.e. the numerator is m*e + (1-m) with e = exp(s/8). Using
m*e + (1-m) = m*(e-1) + 1, the kernel computes
  et = (exp(s/8) - 1) * m          (Act exp from PSUM; DVE sub at 4x,
                                    DVE mult at 2x with a bf16 mask)
and folds the "+1" into the PV matmul as a host-precomputed column-sum
of V (3 tiny rank-1 matmuls per head add colsum(V)+count to PSUM).

Engine budget per core (TimelineSim cost model):
  PE  ~73us  S (27) + PV (15) + QKVO projections (28) + transposes
  Act ~58us  exp only (reads score PSUM directly)
  DVE ~62us  (e-1)*m + normalize + tp copies + q-proj eviction
  Pool ~45us v-proj/k-proj/y evictions (idle engine in baseline)
  DMA ~70us  q,k,v 12MB + mask 8MB (bf16 for the 2x TT) + w 1MB + y 4MB
"""

import math

import numpy as np
import ml_dtypes

BF16 = ml_dtypes.bfloat16
FP8 = ml_dtypes.float8_e4m3
S, H, NH, DK = 2048, 1024, 16, 64
NCORES = 8
HPC = NH // NCORES          # heads per core = 2
DPC = HPC * DK              # head dims per core = 128
KC = H // 128               # contraction chunks = 8
TP = S // 512               # 512-wide token panels = 4
JC = S // 128               # 128-wide key chunks = 16
VA = DK + 1                 # v columns + ones column = 65

_CACHE = {}


def _oslc(ic):
    """o_ps column offset for ic-th 65-wide slice: 7 slices per 512-fp32
    PSUM bank so no matmul crosses a bank boundary."""
    b, r = divmod(ic, 7)
    return b * 512 + r * VA


def _build_program():
    """Build + compile the (identical) per-core Bass program."""
    from contextlib import ExitStack

    import concourse.bacc as bacc
    import concourse.tile as tile
    from concourse import mybir

    dt = mybir.dt
    AF = mybir.ActivationFunctionType
    f8 = dt.float8e4

    nc = bacc.Bacc("TRN2", target_bir_lowering=False, debug=False)

    # token-quarter-major x layouts: [4q][128 p][8 c][512 i] flattened
    qx_d = nc.dram_tensor("qx", [4 * 128, KC * 512], dt.bfloat16, kind="ExternalInput").ap()
    kx_d = nc.dram_tensor("kx", [4 * 128, KC * 512], dt.bfloat16, kind="ExternalInput").ap()
    vx_d = nc.dram_tensor("vx", [4 * 128, KC * 512], dt.bfloat16, kind="ExternalInput").ap()
    maskT_d = nc.dram_tensor("maskT", [S, S], dt.float8e4, kind="ExternalInput").ap()
    wq_d = nc.dram_tensor("wq", [128, KC * DPC], dt.bfloat16, kind="ExternalInput").ap()
    wk_d = nc.dram_tensor("wk", [128, KC * DPC], dt.bfloat16, kind="ExternalInput").ap()
    # wv | wo | ident packed: one DMA for the non-critical weights
    wpk_d = nc.dram_tensor("wpk", [128, 2 * H + 128], dt.bfloat16, kind="ExternalInput").ap()
    bqk_d = nc.dram_tensor("bqk", [DPC, 2], dt.float32, kind="ExternalInput").ap()
    vcr_d = nc.dram_tensor("vcr", [1, HPC * 7 * VA], dt.bfloat16, kind="ExternalInput").ap()
    yT_d = nc.dram_tensor("yT", [H, S], dt.bfloat16, kind="ExternalOutput").ap()

    with tile.TileContext(nc) as tc, ExitStack() as ctx:
        cp = ctx.enter_context(tc.tile_pool(name="const", bufs=1))
        e_p = ctx.enter_context(tc.tile_pool(name="ex", bufs=4))
        ot_p = ctx.enter_context(tc.tile_pool(name="otok", bufs=2))
        rc_p = ctx.enter_context(tc.tile_pool(name="recip", bufs=3))

        # ---- DMA priority: wq, qx quarters (PE-critical), rest behind ----
        wq_sb = cp.tile([128, KC * DPC], dt.bfloat16, tag="wq")
        nc.sync.dma_start(out=wq_sb, in_=wq_d)
        ones_col = cp.tile([1, 128], dt.bfloat16, tag="ones")
        nc.vector.memset(ones_col, 1.0)
        warm = cp.tile([128, 512], dt.bfloat16, tag="warm")
        nc.vector.memset(warm, 0.0)

        qT_sb = cp.tile([128, S], dt.bfloat16, tag="qTs")
        kT_sb = cp.tile([128, S], dt.bfloat16, tag="kTs")
        vaug = cp.tile([128, JC * (HPC * VA)], dt.bfloat16, tag="vaug")
        oT_sb = [cp.tile([128, 512], dt.bfloat16, tag=f"oTp{p}", name=f"oTp{p}")
                 for p in range(TP)]
        # y pair-tiles: cols 0:512 = even nn, 512:1024 = odd nn (one panel)
        y_p = ctx.enter_context(tc.tile_pool(name="ysb", bufs=2))

        import concourse.bass as bass_mod

        # tiny inputs first: they gate the projection evictions
        bqk_sb = cp.tile([DPC, 2], dt.float32, tag="bqk")
        nc.sync.dma_start(out=bqk_sb, in_=bqk_d)
        bq_sb = bqk_sb[:, 0:1]
        bk_sb = bqk_sb[:, 1:2]
        vcr_sb = cp.tile([1, HPC * 7 * VA], dt.bfloat16, tag="vcr")
        nc.sync.dma_start(out=vcr_sb, in_=vcr_d)
        etab = cp.tile([1, 2], dt.bfloat16, tag="etab")
        nc.scalar.activation(etab, ones_col[:, 0:2], AF.Exp)

        kv_pool = ctx.enter_context(tc.tile_pool(name="kvin", bufs=1))
        kin = [None] * TP
        vin = [None] * TP

        def x_quarter(which, qq):
            d, store, pre = {
                "k": (kx_d, kin, "k"), "v": (vx_d, vin, "v"),
            }[which]
            t_ = kv_pool.tile([128, KC * 512], dt.bfloat16, tag=f"x{pre}{qq}",
                              name=f"x{pre}{qq}")
            nc.sync.dma_start(out=t_, in_=d[qq * 128:(qq + 1) * 128, :])
            store[qq] = t_

        with tc.tile_pool(name="qxin", bufs=2) as qx_pool, \
             tc.tile_pool(name="ps_proj", bufs=1, space="PSUM") as pq:
            qin = [None] * TP

            def q_quarter(qq):
                t_ = qx_pool.tile([128, KC * 512], dt.bfloat16, tag="xq",
                                  name=f"xq{qq}")
                nc.sync.dma_start(out=t_, in_=qx_d[qq * 128:(qq + 1) * 128, :])
                qin[qq] = t_

            q_quarter(0)
            q_quarter(1)
            wk_sb = cp.tile([128, KC * DPC], dt.bfloat16, tag="wk")
            nc.sync.dma_start(out=wk_sb, in_=wk_d)
            x_quarter("k", 0)
            q_quarter(2)
            q_quarter(3)
            wpk = cp.tile([128, 2 * H + 128], dt.bfloat16, tag="wpk")
            nc.sync.dma_start(out=wpk, in_=wpk_d)
            w_sb = {"wq": wq_sb, "wk": wk_sb, "wv": wpk[:, 0:H]}
            wo_sb = wpk[:, H:2 * H]
            ident = wpk[:, 2 * H:2 * H + 128]
            # warm up the PE p-state ramp while the first q quarter lands
            wps = pq.tile([128, 512], dt.float32, tag="warmp", name="warmp")
            for i in range(10):
                nc.tensor.matmul(
                    wps, lhsT=warm[:, 0:128], rhs=warm,
                    start=(i == 0), stop=(i == 9),
                )
            # vaug ones-columns: one strided memset (col 64 of every 65-slice)
            vaug_ones = bass_mod.AP(
                tensor=vaug.tensor,
                offset=vaug.offset + DK,
                ap=[vaug.ap[0], [VA, JC * HPC]],
            )
            nc.vector.memset(vaug_ones, 1.0)

            def q_proj_panel(p):
                ps = pq.tile([128, 512], dt.float32, tag=f"pq{p}", name=f"pq{p}")
                for kk in range(KC):
                    nc.tensor.matmul(
                        ps,
                        lhsT=w_sb["wq"][:, kk * DPC:(kk + 1) * DPC],
                        rhs=qin[p][:, kk * 512:(kk + 1) * 512],
                        start=(kk == 0),
                        stop=(kk == KC - 1),
                    )
                nc.vector.tensor_scalar_add(
                    qT_sb[:, p * 512:(p + 1) * 512], ps, bq_sb
                )

            kps = pq.tile([128, 512], dt.float32, tag="pk0", name="pk0")

            def k0_proj():
                for kk in range(KC):
                    nc.tensor.matmul(
                        kps,
                        lhsT=w_sb["wk"][:, kk * DPC:(kk + 1) * DPC],
                        rhs=kin[0][:, kk * 512:(kk + 1) * 512],
                        start=(kk == 0),
                        stop=(kk == KC - 1),
                    )
                nc.vector.tensor_scalar_add(kT_sb[:, 0:512], kps, bk_sb)

            q_proj_panel(0)
            q_proj_panel(1)
            k0_proj()

        # ---- mask: fp8 DMA into the upper byte-half of each block's
        # bf16 slot; the idle Pool engine converts in place (the write
        # pointer trails the read pointer, so forward streaming is safe).
        # h0's TT reads the fp8 view at DVE 1x (h0 is DMA-paced anyway);
        # h1 reads the converted bf16 at 2x. The conversion is emitted at
        # h0's iteration j, after h0's TT(j) consumed the fp8 view. ----
        mask_lo = cp.tile([128, 8 * S], dt.bfloat16, tag="mask_lo")
        mask_hi = cp.tile([128, 8 * S], dt.bfloat16, tag="mask_hi")

        def mask_ap(j):
            t_ = mask_lo if j < 8 else mask_hi
            return t_[:, (j % 8) * S:(j % 8 + 1) * S]

        def mask8_ap(j):
            t_ = (mask_lo if j < 8 else mask_hi).bitcast(f8)
            return t_[:, (j % 8) * 2 * S + S:(j % 8) * 2 * S + 2 * S]

        def mask_pair(j):
            t_ = (mask_lo if j < 8 else mask_hi).bitcast(f8)
            nc.sync.dma_start(
                out=t_[:, (j % 8) * 2 * S + S:(j % 8 + 2) * 2 * S].rearrange(
                    "p (a i) -> p a i", a=2
                )
                if False else bass_mod.AP(
                    tensor=t_.tensor,
                    offset=t_.offset + (j % 8) * 2 * S + S,
                    ap=[t_.ap[0], [2 * S, 2], [1, S]],
                ),
                in_=maskT_d[j * 128:(j + 2) * 128, :].rearrange(
                    "(a p) i -> p a i", p=128
                ),
            )

        for tok in ("m0 k1 v0 m2 m4 v1 k2 m6 m8 v2 k3 m10 m12 v3 "
                    "m14").split():
            if tok[0] == "m":
                mask_pair(int(tok[1:]))
            else:
                x_quarter(tok[0], int(tok[1:]))

        # ---- attention; V projection and just-in-time K-panel projections
        # ride the h=0 j-loop. PSUM banks: s 2x2 + o 3 + misc 1 = 8
        if True:
            with tc.tile_pool(name="ps_misc", bufs=1, space="PSUM") as pm, \
                 tc.tile_pool(name="ps_s", bufs=2, space="PSUM") as ps_p, \
                 tc.tile_pool(name="ps_o", bufs=1, space="PSUM") as po_p:

                def q_proj_late(p):
                    ps = pm.tile([128, 512], dt.float32, tag="misc",
                                 name=f"pq{p}")
                    for kk in range(KC):
                        nc.tensor.matmul(
                            ps[:, 0:512],
                            lhsT=w_sb["wq"][:, kk * DPC:(kk + 1) * DPC],
                            rhs=qin[p][:, kk * 512:(kk + 1) * 512],
                            start=(kk == 0),
                            stop=(kk == KC - 1),
                        )
                    nc.vector.tensor_scalar_add(
                        qT_sb[:, p * 512:(p + 1) * 512], ps[:, 0:512], bq_sb
                    )

                def k_proj_panel(p):
                    ps = pm.tile([128, 512], dt.float32, tag="misc", name=f"pk{p}")
                    for kk in range(KC):
                        nc.tensor.matmul(
                            ps[:, 0:512],
                            lhsT=w_sb["wk"][:, kk * DPC:(kk + 1) * DPC],
                            rhs=kin[p][:, kk * 512:(kk + 1) * 512],
                            start=(kk == 0),
                            stop=(kk == KC - 1),
                        )
                    nc.vector.tensor_scalar_add(
                        kT_sb[:, p * 512:(p + 1) * 512], ps[:, 0:512], bk_sb
                    )

                def v_proj_chunk(t):
                    """Token-chunk t of the V projection into vaug (Act evicts)."""
                    ps = pm.tile([128, 512], dt.float32, tag="misc", name=f"pv{t}")
                    qq, ts_ = divmod(t, 4)
                    for kk in range(KC):
                        nc.tensor.matmul(
                            ps[:, 0:DPC],
                            lhsT=vin[qq][:, kk * 512 + ts_ * 128: kk * 512 + (ts_ + 1) * 128],
                            rhs=w_sb["wv"][:, kk * DPC:(kk + 1) * DPC],
                            start=(kk == 0),
                            stop=(kk == KC - 1),
                        )
                    base = t * (HPC * VA)
                    vdst = bass_mod.AP(
                        tensor=vaug.tensor,
                        offset=vaug.offset + base,
                        ap=[vaug.ap[0], [VA, HPC], [1, DK]],
                    )
                    nc.scalar.copy(vdst, ps[:, 0:DPC].rearrange(
                        "p (a d) -> p a d", a=HPC))

                def pv_mms(h, j, et, o_ps):
                    for ic in range(JC):
                        nc.tensor.matmul(
                            o_ps[:, _oslc(ic): _oslc(ic) + VA],
                            lhsT=et[:, ic * 128:(ic + 1) * 128],
                            rhs=vaug[:, j * (HPC * VA) + h * VA: j * (HPC * VA) + (h + 1) * VA],
                            start=(j == 0 and ic % 7 == 0),
                            stop=False,
                        )

                deferred = []  # [hs, ot_big, next_ic] transpose drip

                def drip_one():
                    if not deferred or deferred[0][2] >= JC:
                        return
                    dhs, dot, ic = deferred[0]
                    deferred[0][2] += 1
                    tp = pm.tile([DK, 128], dt.bfloat16, tag="misc",
                                 name=f"dtp{ic}")
                    nc.tensor.transpose(tp, dot[:, ic * DK:(ic + 1) * DK], ident)
                    nc.vector.tensor_copy(
                        oT_sb[ic // 4][dhs:dhs + DK,
                                       (ic % 4) * 128:(ic % 4 + 1) * 128],
                        tp,
                    )

                def epilogue(h, o_ps):
                    hs = h * DK
                    # +1 correction: colsum(V)+count via rank-1 matmuls,
                    # one per 65-wide slice (stop closes each PSUM bank)
                    for ic in range(JC):
                        nc.tensor.matmul(
                            o_ps[:, _oslc(ic): _oslc(ic) + VA],
                            lhsT=ones_col,
                            rhs=vcr_sb[:, h * 7 * VA + (ic % 7) * VA:
                                       h * 7 * VA + (ic % 7) * VA + VA],
                            start=False,
                            stop=(ic in (6, 13, 15)),
                        )
                    ot_big = ot_p.tile([128, JC * DK], dt.bfloat16, tag="ot")
                    if h < HPC - 1:
                        deferred.append([hs, ot_big, 0])
                    for b in range(3):
                        n_ic = (7, 7, 2)[b]
                        rc = rc_p.tile([128, 8], dt.float32, tag="rc",
                                       name=f"rc{h}_{b}")
                        den = bass_mod.AP(
                            tensor=o_ps.tensor,
                            offset=o_ps.offset + b * 512 + DK,
                            ap=[o_ps.ap[0], [VA, n_ic]],
                        )
                        nc.vector.reciprocal(rc[:, :n_ic], den)
                        src_ap = bass_mod.AP(
                            tensor=o_ps.tensor,
                            offset=o_ps.offset + b * 512,
                            ap=[o_ps.ap[0], [VA, n_ic], [1, DK]],
                        )
                        rcb = bass_mod.AP(
                            tensor=rc.tensor,
                            offset=rc.offset,
                            ap=[rc.ap[0], [1, n_ic], [0, DK]],
                        )
                        dst = ot_big[:, b * 7 * DK:(b * 7 + n_ic) * DK].rearrange(
                            "p (a d) -> p a d", d=DK
                        )
                        nc.vector.tensor_mul(dst, src_ap, rcb)
                        if h < HPC - 1:
                            continue
                        # tail: transpose/copy this bank's slices
                        for ic in range(b * 7, b * 7 + n_ic):
                            ot = ot_big[:, ic * DK:(ic + 1) * DK]
                            if ic % 2 == 0:
                                tp = ps_p.tile([DK, 128], dt.bfloat16,
                                               tag="sps", name=f"tp{h}_{ic}")
                            else:
                                tp = pm.tile([DK, 128], dt.bfloat16,
                                             tag="misc", name=f"tp{h}_{ic}")
                            nc.tensor.transpose(tp, ot, ident)
                            dsto = oT_sb[ic // 4][hs:hs + DK,
                                                  (ic % 4) * 128:(ic % 4 + 1) * 128]
                            if ic % 2 == 0:
                                nc.vector.tensor_copy(dsto, tp)
                            else:
                                nc.scalar.copy(dsto, tp)
                        for p in ((0,), (1, 2), (3,))[b]:
                            y_panel(p)

                def y_panel(p):
                    # O-projection: 4 pair-matmuls per panel into a 3-deep
                    # rotation of PSUM pair-buffers (2x "sps" + 1 slice of
                    # the idle o banks); evictions split across Act and DVE
                    for pr in range(4):
                        gp = p * 4 + pr
                        if gp % 3 == 2:
                            yb = po_p.tile([128, 1536], dt.float32, tag="ops",
                                           name=f"ybig{gp}")
                            y_ps = yb[:, 0:1024]
                        else:
                            y_ps = ps_p.tile([128, 1024], dt.float32,
                                             tag="sps", name=f"y{p}_{pr}")
                        y_t = y_p.tile([128, 1024], dt.bfloat16,
                                       tag=f"ysb{pr}", name=f"y_t{p}_{pr}")
                        for e in range(2):
                            nc.tensor.matmul(
                                y_ps[:, e * 512:(e + 1) * 512],
                                lhsT=wo_sb[:, (2 * pr + e) * 128:
                                           (2 * pr + e + 1) * 128],
                                rhs=oT_sb[p],
                                start=True,
                                stop=True,
                            )
                        nc.scalar.copy(y_t[:, 0:512], y_ps[:, 0:512])
                        nc.vector.tensor_copy(y_t[:, 512:1024],
                                              y_ps[:, 512:1024])
                        nc.sync.dma_start(
                            out=yT_d[2 * pr * 128:(2 * pr + 2) * 128,
                                     p * 512:(p + 1) * 512
                                     ].rearrange("(a p) i -> p a i", p=128),
                            in_=y_t.rearrange("p (a i) -> p a i", a=2),
                        )

                # flat 32-step stream over (head, key-block): the software
                # pipeline (depth 2) carries across the head boundary so
                # h1's S/exp/mask run while h0's epilogue drains
                pend = []
                o_cur = [None, None]  # (head, o_ps tile)
                for gi in range(HPC * JC + 2):
                    if len(pend) == 2 or gi >= HPC * JC:
                        hd, jd, etd = pend.pop(0)
                        if o_cur[0] != hd:
                            o_cur = [hd, po_p.tile([128, 1536], dt.float32,
                                                   tag="ops", name=f"ops{hd}")]
                        if hd == 0:
                            v_proj_chunk(jd)
                        pv_mms(hd, jd, etd, o_cur[1])
                        if jd == JC - 1:
                            epilogue(hd, o_cur[1])
                    if gi >= HPC * JC:
                        continue
                    h, j = divmod(gi, JC)
                    hs = h * DK
                    et = e_p.tile([128, S], dt.bfloat16, tag="et")
                    s_tiles = []
                    for half in range(2):
                        s_ps = ps_p.tile([128, 1024], dt.float32, tag="sps")
                        s_tiles.append(s_ps)
                        for q in range(2):
                            pi = half * 2 + q
                            nc.tensor.matmul(
                                s_ps[:, q * 512:(q + 1) * 512],
                                lhsT=kT_sb[hs:hs + DK, j * 128:(j + 1) * 128],
                                rhs=qT_sb[hs:hs + DK, pi * 512:(pi + 1) * 512],
                                start=True,
                                stop=True,
                            )
                        if gi == 0 and half == 0:
                            # late q panels: S(j0,half0) only needs panels
                            # 0-1, so panels 2-3 project while it runs
                            q_proj_late(2)
                            q_proj_late(3)
                    a, bt = s_tiles
                    if bt.offset == a.offset + 1024 and bt.tensor is a.tensor:
                        # adjacent ring slots: one 2048-wide exp + sub
                        s_all = bass_mod.AP(
                            tensor=a.tensor, offset=a.offset,
                            ap=[a.ap[0], [1, 2048]],
                        )
                        nc.scalar.activation(et, s_all, AF.Exp,
                                             scale=1.0 / math.sqrt(DK))
                        nc.vector.tensor_scalar_sub(et, et, 1.0)
                    else:
                        for half in range(2):
                            eh = et[:, half * 1024:(half + 1) * 1024]
                            nc.scalar.activation(eh, s_tiles[half], AF.Exp,
                                                 scale=1.0 / math.sqrt(DK))
                            nc.vector.tensor_scalar_sub(eh, eh, 1.0)
                    if h == 0:
                        nc.vector.tensor_mul(et, et, mask8_ap(j))
                        nc.gpsimd.tensor_copy(mask_ap(j), mask8_ap(j))
                    else:
                        nc.vector.tensor_mul(et, et, mask_ap(j))
                    if h == 0 and j in (2, 6, 10):
                        k_proj_panel(j // 4 + 1)

                    if h == 1:
                        drip_one()
                        if gi >= 30:
                            drip_one()
                    pend.append((h, j, et))

    nc.compile()
    return nc


def get_program():
    if "nc" not in _CACHE:
        _CACHE["nc"] = _build_program()
    return _CACHE["nc"]


def _wshuf(wT):
    """[1024 k, 128 n] -> [128 p, KC*128] with chunk kk at cols kk*128."""
    return np.ascontiguousarray(
        wT.reshape(KC, 128, DPC).transpose(1, 0, 2).reshape(128, KC * DPC)
    ).astype(BF16)


def _xquarters(x):
    """[S tok, H feat] fp32 -> [4*128, 8*512] bf16 token-quarter-major:
    [q][p][c][i] with element = x[512q + i, 128c + p]."""
    xT = np.asarray(x, np.float32).T            # [H, S]
    x4 = xT.reshape(KC, 128, TP, 512)           # [c, p, q, i]
    return np.ascontiguousarray(
        x4.transpose(2, 1, 0, 3).reshape(TP * 128, KC * 512)
    ).astype(BF16)


def make_in_maps(query, key, value, attention_mask, Wq, bq, Wk, bk, Wv, Wo):
    """Host-side sharding: per-core input dicts."""
    qx = _xquarters(np.asarray(query, np.float32)[0])
    kx = _xquarters(np.asarray(key, np.float32)[0])
    vx = _xquarters(np.asarray(value, np.float32)[0])
    maskT = np.ascontiguousarray(
        np.asarray(attention_mask, np.float32)[0, 0].T
    ).astype(FP8)
    # colsum(V)+count per core-head, device-matched: colsum over bf16(V)
    v_f = np.asarray(value, np.float32)[0]
    Wv_f = np.asarray(Wv, np.float32)

    in_maps = []
    for c in range(NCORES):
        ns = slice(c * DPC, (c + 1) * DPC)
        vproj = (v_f @ Wv_f[ns].T).astype(BF16).astype(np.float32)  # [S, 128]
        vcol = vproj.sum(axis=0)                                    # [128]
        vcr = np.zeros((1, HPC * 7 * VA), np.float32)
        for h in range(HPC):
            tile65 = np.concatenate([vcol[h * DK:(h + 1) * DK], [float(S)]])
            vcr[0, h * 7 * VA:(h + 1) * 7 * VA] = np.tile(tile65, 7)
        wpk = np.concatenate(
            [
                _wshuf(Wv_f[ns].T),
                np.ascontiguousarray(np.asarray(Wo, np.float32)[:, ns].T).astype(BF16),
                np.eye(128, dtype=BF16),
            ],
            axis=1,
        )
        bqk = np.stack(
            [np.asarray(bq, np.float32)[ns], np.asarray(bk, np.float32)[ns]],
            axis=1,
        )
        in_maps.append(
            {
                "qx": qx,
                "kx": kx,
                "vx": vx,
                "maskT": maskT,
                "wq": _wshuf(np.asarray(Wq, np.float32)[ns].T),
                "wk": _wshuf(np.asarray(Wk, np.float32)[ns].T),
                "wpk": np.ascontiguousarray(wpk),
                "bqk": np.ascontiguousarray(bqk),
                "vcr": vcr.astype(BF16),
            }
        )
    return in_maps


def combine_outputs(results, Wv_bias, Wo, bo):
    """Sum per-core partial yT's (bf16 -> fp32), add host-folded biases."""
    acc = np.zeros((H, S), np.float32)
    for r in results:
        acc += r["yT"].astype(np.float32)
    bias = np.asarray(bo, np.float32) + np.asarray(Wv_bias, np.float32) @ np.asarray(
        Wo, np.float32
    ).T
    return (acc.T + bias[None, :]).astype(np.float32)[None]


def kernel(
    query,
    key,
    value,
    attention_mask,
    Wq,
    bq,
    Wk,
    bk,
    Wv,
    bv,
    Wo,
    bo,
    head,
    hidden_size,
):
    from concourse.bass_utils import run_bass_kernel_spmd

    nc = get_program()
    in_maps = make_in_maps(
        query, key, value, attention_mask, Wq, bq, Wk, bk, Wv, Wo
    )
    res = run_bass_kernel_spmd(nc, in_maps, list(range(NCORES)))
    return combine_outputs(res.results, bv, Wo, bo)
